# revision 44
# baseline (speedup 1.0000x reference)
"""AttentiveGraphConvolution (GAT-style layer) on 8 trn2 NeuronCores.

Math (reference):
    h   = x @ W                       [N, D]
    a_s = h @ attn_self               [N, 1]
    a_n = h @ attn_neigh              [N, 1]
    e   = leaky_relu(a_s + a_n.T, 0.2)
    e   = e + NEG_INF * (1 - adj)
    out = relu(softmax(e, -1) @ h)

Reformulation used here (exact in fp32 up to rounding):
    s_ij = a_s[i] + a_n[j]
    exp(leaky(s)) = exp(0.2 s) * max(exp(0.8 s), 1)       (leaky alpha = 0.2)
    exp(0.8 s)    = w[i] * w2[j],  w = e^{0.8 a_s}, w2 = e^{0.8 a_n}
    adj binary =>  masked weight t_ij = adj_ij * u2_i * v2_j * max(w_i w2_j, 1)

    out_i = relu( (sum_j t_ij h_j) / (sum_j t_ij) )
          = relu( (sum_j q_ji h2_j) / (sum_j q_ji v2_j) )   (u2_i cancels)
    with q_ji  = adjT_ji * max(w_i w2_j, 1)                 [j, i] layout
         h2_j  = v2_j * h_j

Per adj tile the device work is:  R = w2_j * W_bcast  (ACT copy-with-scale),
q = (R max 1) * adjT  (DVE scalar_tensor_tensor), then accumulating float32r
matmuls  outT += h2_chunk.T @ q  and  rs += v2_chunk.T @ q  on the PE.

Changes vs the first-pass kernel (profile-driven):
  * ONE collective instead of two.  Each NRT collective costs ~20 us of
    serial CC-stream time on top of the launch-stagger rendezvous, and the
    trace showed a further 33 us gpsimd stall between the a_n AllGather and
    the h2 AllGather trigger.  a_n now rides in the same gathered tensor as
    h2 (8 extra rows appended to the 1024-row partition-major h2 block).
  * a_s/a_n are computed straight from x via av2 = [W@attn_self|W@attn_neigh]
    (a [2,512] matmul then av = av2T @ x), so h2 production and the gather
    trigger no longer wait for the full hT pass.
  * x and W ship as bf16 (a_v/h accumulate in fp32 PSUM; rel-err stays ~5e-3,
    well inside the 2e-2 gate), halving the phase-1 DMA.
  * The gathered h2 lands in 8 per-source-core tiles so the first matmul
    only waits on block 0's read-back, not all eight.
The main accumulation loop is untouched from the validated baseline.

Sharding: output rows across 8 cores. Each core receives its adj row-slab as
bf16 (adj is binary so bf16 is exact), pre-transposed and row-interleaved in
groups of GP=4 (host layout choice) so each DMA descriptor covers 4 adjacency
rows = 8 KB contiguous.
"""

import numpy as np

N = 8192
DIN = 512
DOUT = 128
NCORES = 8
S = N // NCORES     # 1024 output rows per core
GP = 4              # adjacency rows per partition per DMA (descriptor size)


def _emit(nc, tc, ctx, n, s, din, dout):
    from concourse import masks, mybir

    f32 = mybir.dt.float32
    f32r = mybir.dt.float32r
    bf16 = mybir.dt.bfloat16
    AF = mybir.ActivationFunctionType
    ALU = mybir.AluOpType

    P = 128
    jc_n = n // P       # j chunks over all nodes
    sc_n = s // P       # chunks in the local row slab
    kc_n = din // P     # contraction chunks for x @ W
    nb = min(512, s)    # matmul moving-dim block
    ib_n = s // nb      # i blocks per core (free dim of main matmuls)
    g_n = jc_n // GP    # adj super-chunks (GP j-chunks per DMA)
    sr = s + sc_n       # gathered rows per core: 1024 h2 + 8 a_n rows

    adjt = nc.dram_tensor("adjt", [n, s], bf16, kind="ExternalInput")
    xt = nc.dram_tensor("xt", [din, s], bf16, kind="ExternalInput")
    wmat = nc.dram_tensor("wmat", [din, dout], bf16, kind="ExternalInput")
    wt = nc.dram_tensor("wt", [dout, din], bf16, kind="ExternalInput")
    att = nc.dram_tensor("att", [dout, 2], bf16, kind="ExternalInput")
    out = nc.dram_tensor("out", [s, dout], f32, kind="ExternalOutput")

    const_pool = ctx.enter_context(tc.tile_pool(name="const", bufs=1))
    ph1_pool = ctx.enter_context(tc.tile_pool(name="ph1", bufs=1))
    ph1_psum = ctx.enter_context(tc.tile_pool(name="ph1_psum", bufs=1, space="PSUM"))
    tp_psum = ctx.enter_context(tc.tile_pool(name="tp_psum", bufs=2, space="PSUM"))
    acc_psum = ctx.enter_context(tc.tile_pool(name="acc_psum", bufs=1, space="PSUM"))
    dram_pool = ctx.enter_context(tc.tile_pool(name="dram", bufs=1, space="DRAM"))
    adj_pool = ctx.enter_context(tc.tile_pool(name="adj", bufs=6))
    r_pool = ctx.enter_context(tc.tile_pool(name="r", bufs=5))
    q_pool = ctx.enter_context(tc.tile_pool(name="q", bufs=8))
    fin_pool = ctx.enter_context(tc.tile_pool(name="fin", bufs=2))

    ident = const_pool.tile([P, P], f32, name="ident")
    masks.make_identity(nc, ident[:])
    identb = const_pool.tile([P, P], bf16, name="identb")
    nc.scalar.activation(identb[:], ident[:], AF.Copy)
    identr = const_pool.tile([P, P], f32r, name="identr")
    nc.scalar.activation(identr[:], ident[:], AF.Copy)

    # ---- Phase 1: input DMAs, attention vectors straight from x ----------
    w_sb = []
    x_sb = []
    for k in range(kc_n):
        wk = ph1_pool.tile([P, dout], bf16, name="w_sb", tag=f"w_sb{k}")
        nc.sync.dma_start(wk[:], wmat[k * P:(k + 1) * P, :])
        w_sb.append(wk)
        xk = ph1_pool.tile([P, s], bf16, name="x_sb", tag=f"x_sb{k}")
        nc.sync.dma_start(xk[:], xt[k * P:(k + 1) * P, :])
        x_sb.append(xk)
    wt_sb = ph1_pool.tile([P, din], bf16, name="wt_sb")
    nc.sync.dma_start(wt_sb[:], wt[:])
    att_sb = const_pool.tile([P, 2], bf16, name="att_sb")
    nc.sync.dma_start(att_sb[:], att[:])

    # av2 = [W@attn_self | W@attn_neigh].T : [2, din]
    av2_ps = tp_psum.tile([2, din], f32, name="av2_ps", tag="tp")
    nc.tensor.matmul(av2_ps[:], att_sb[:], wt_sb[:], start=True, stop=True)
    av2_sb = ph1_pool.tile([2, din], bf16, name="av2_sb")
    nc.scalar.activation(av2_sb[:], av2_ps[:], AF.Copy)
    av2T_sb = []
    for k in range(kc_n):
        avT_ps = tp_psum.tile([P, 2], bf16, name="avT_ps", tag="tp")
        nc.tensor.matmul(
            avT_ps[:], av2_sb[:, k * P:(k + 1) * P], identb[:2, :2],
            is_transpose=True, start=True, stop=True,
        )
        a2t = ph1_pool.tile([P, 2], bf16, name="av2T_sb", tag=f"av2T{k}")
        nc.scalar.activation(a2t[:], avT_ps[:], AF.Copy)
        av2T_sb.append(a2t)
    # av[2, s] = [a_s ; a_n] for the local slab, straight from x
    av_sb = ph1_pool.tile([2, s], f32r, name="av_sb")
    for b in range(ib_n):
        avl_ps = tp_psum.tile([2, nb], f32, name="avl_ps", tag="tp")
        for k in range(kc_n):
            nc.tensor.matmul(
                avl_ps[:], av2T_sb[k][:], x_sb[k][:, b * nb:(b + 1) * nb],
                start=(k == 0), stop=(k == kc_n - 1),
            )
        nc.scalar.activation(av_sb[:, b * nb:(b + 1) * nb], avl_ps[:], AF.Copy)

    # W_bcast[p, i] = exp(0.8 * a_s_local[i]) for every partition p
    wrow_sb = ph1_pool.tile([1, s], f32, name="wrow_sb")
    nc.scalar.activation(wrow_sb[:], av_sb[0:1, :], AF.Exp, scale=0.8)
    ones_sb = const_pool.tile([1, P], f32, name="ones_sb")
    nc.gpsimd.memset(ones_sb[:], 1.0)
    wb_sb = const_pool.tile([P, s], f32, name="wb_sb")
    for b in range(ib_n):
        wb_ps = tp_psum.tile([P, nb], f32, name="wb_ps", tag="tp")
        nc.tensor.matmul(
            wb_ps[:], ones_sb[:], wrow_sb[:, b * nb:(b + 1) * nb],
            start=True, stop=True,
        )
        nc.scalar.activation(wb_sb[:, b * nb:(b + 1) * nb], wb_ps[:], AF.Copy)

    # hT[d, n_local] = (x @ W).T for the local slab
    hT_sb = ph1_pool.tile([P, s], f32, name="hT_sb")
    for b in range(ib_n):
        hT_ps = ph1_psum.tile([P, nb], f32, name="hT_ps")
        for k in range(kc_n):
            nc.tensor.matmul(
                hT_ps[:],
                w_sb[k][:],
                x_sb[k][:, b * nb:(b + 1) * nb],
                start=(k == 0), stop=(k == kc_n - 1),
            )
        nc.scalar.activation(hT_sb[:, b * nb:(b + 1) * nb], hT_ps[:], AF.Copy)

    # ---- Phase 2: h2 shard + a_n packed into ONE gathered tensor ---------
    # Local chunk c is written to rows {p*sc_n + c} so the gathered tensor
    # reads back with 4 KB-contiguous per-partition descriptors; rows
    # s..s+sc_n-1 carry a_n for the local slab (row t = nodes t*128..).
    anT_sb = ph1_pool.tile([P, sc_n], f32, name="anT_sb")
    for c in range(sc_n):
        avT2_ps = tp_psum.tile([P, 2], f32r, name="avT2_ps", tag="tp")
        nc.tensor.matmul(
            avT2_ps[:], av_sb[:, c * P:(c + 1) * P], identr[:2, :2],
            is_transpose=True, start=True, stop=True,
        )
        nc.scalar.activation(anT_sb[:, c:c + 1], avT2_ps[:, 1:2], AF.Copy)
    v2loc_sb = ph1_pool.tile([P, sc_n], f32, name="v2loc_sb")
    nc.scalar.activation(v2loc_sb[:], anT_sb[:], AF.Exp, scale=0.2)

    h2an_dram = dram_pool.tile([sr, dout], f32r, name="h2an_dram")
    h2an_pm = h2an_dram[0:s].rearrange("(p kl) d -> kl p d", kl=sc_n)
    for c in range(sc_n):
        hn_ps = tp_psum.tile([P, P], f32, name="hn_ps", tag="tp")
        nc.tensor.matmul(
            hn_ps[:], hT_sb[:, c * P:(c + 1) * P], ident[:],
            is_transpose=True, start=True, stop=True,
        )
        h2c_sb = fin_pool.tile([P, dout], f32r, name="h2c_sb")
        nc.scalar.activation(h2c_sb[:], hn_ps[:], AF.Copy, scale=v2loc_sb[:, c:c + 1])
        nc.sync.dma_start(h2an_pm[c], h2c_sb[:])
    # a_n rows: flatten the 8x128 tail rows into one [1, 1024] view
    nc.sync.dma_start(
        h2an_dram[s:sr].rearrange("r p -> (r p)")[None, :], av_sb[1:2, :])

    groups = [list(range(NCORES))]
    h2full_dram = dram_pool.tile([NCORES * sr, dout], f32r, addr_space="Shared",
                                 name="h2full")
    nc.gpsimd.collective_compute(
        "AllGather", ALU.bypass, replica_groups=groups,
        ins=[h2an_dram.opt()], outs=[h2full_dram.opt()],
    )

    # ---- Phase 3: unpack gathered a_n -> w2/v2, h2 block tiles -----------
    anf_raw = ph1_pool.tile([jc_n, P], f32r, name="anf_raw")
    for cc in range(NCORES):
        nc.sync.dma_start(
            anf_raw[cc * sc_n:(cc + 1) * sc_n, :],
            h2full_dram[cc * sr + s:(cc + 1) * sr, :],
        )
    anf_ps = tp_psum.tile([P, jc_n], f32r, name="anf_ps", tag="tp")
    nc.tensor.matmul(anf_ps[:], anf_raw[:], identr[:jc_n, :jc_n],
                     is_transpose=True, start=True, stop=True)
    w2_sb = const_pool.tile([P, jc_n], f32, name="w2_sb")
    nc.scalar.activation(w2_sb[:], anf_ps[:], AF.Exp, scale=0.8)
    v2f_sb = const_pool.tile([P, jc_n], f32r, name="v2f_sb")
    nc.scalar.activation(v2f_sb[:], anf_ps[:], AF.Exp, scale=0.2)

    # gathered h2 as 8 per-source-core tiles (first matmul waits on block 0
    # only); 4 KB per-partition descriptors via the partition-major layout
    h2blk = []
    for cc in range(NCORES):
        hb = ph1_pool.tile([P, sc_n * dout], f32r, name="h2blk", tag=f"h2b{cc}")
        nc.sync.dma_start(
            hb[:],
            h2full_dram[cc * sr:cc * sr + s, :].rearrange(
                "(p kl) d -> p (kl d)", kl=sc_n),
        )
        h2blk.append(hb)

    # adjacency stream (whole slab, ring of 6 super-chunks)
    adj_t = []
    for g in range(g_n):
        at = adj_pool.tile([P, GP * s], bf16, name="adj_t")
        nc.sync.dma_start(
            at[:],
            adjt[g * GP * P:(g + 1) * GP * P, :].rearrange(
                "(p r) i -> p (r i)", r=GP),
        )
        adj_t.append(at)

    # ---- Phase 4: main loop over adj super-chunks (unchanged) ------------
    mm_ps = [acc_psum.tile([P, nb], f32, name=f"mm_ps{b}") for b in range(ib_n)]
    rs_ps = [acc_psum.tile([1, nb], f32, name=f"rs_ps{b}") for b in range(ib_n)]
    for g in range(g_n):
        for r in range(GP):
            j = g * GP + r
            r_t = r_pool.tile([P, s], f32, name="r_t")
            nc.scalar.activation(r_t[:], wb_sb[:], AF.Copy, scale=w2_sb[:, j:j + 1])
            q_t = q_pool.tile([P, s], f32r, name="q_t")
            nc.vector.scalar_tensor_tensor(
                q_t[:], r_t[:], 1.0, adj_t[g][:, r * s:(r + 1) * s],
                op0=ALU.max, op1=ALU.mult,
            )
            st = h2blk[j // sc_n][:, (j % sc_n) * dout:(j % sc_n + 1) * dout]
            for b in range(ib_n):
                nc.tensor.matmul(
                    mm_ps[b][:], st, q_t[:, b * nb:(b + 1) * nb],
                    start=(j == 0), stop=(j == jc_n - 1),
                )
            for b in range(ib_n):
                nc.tensor.matmul(
                    rs_ps[b][:], v2f_sb[:, j:j + 1], q_t[:, b * nb:(b + 1) * nb],
                    start=(j == 0), stop=(j == jc_n - 1),
                )

    # ---- Phase 5: normalize, relu, transpose out -------------------------
    rs_sb = ph1_pool.tile([1, s], f32, name="rs_sb")
    for b in range(ib_n):
        nc.scalar.activation(rs_sb[:, b * nb:(b + 1) * nb], rs_ps[b][:], AF.Copy)
    rs_dram = dram_pool.tile([sc_n, P], f32, name="rs_dram")
    nc.sync.dma_start(rs_dram[:].rearrange("k p -> (k p)")[None, :], rs_sb[0:1, :])
    rs_raw = ph1_pool.tile([sc_n, P], f32, name="rs_raw")
    nc.sync.dma_start(rs_raw[:], rs_dram[:])
    rsT_ps = tp_psum.tile([P, sc_n], f32, name="rsT_ps", tag="tp")
    nc.tensor.matmul(rsT_ps[:], rs_raw[:], ident[:sc_n, :sc_n],
                     is_transpose=True, start=True, stop=True)
    rrT_sb = ph1_pool.tile([P, sc_n], f32, name="rrT_sb")
    nc.vector.reciprocal(rrT_sb[:], rsT_ps[:])

    mo_sb = ph1_pool.tile([P, s], f32, name="mo_sb")
    for b in range(ib_n):
        nc.scalar.activation(mo_sb[:, b * nb:(b + 1) * nb], mm_ps[b][:], AF.Copy)
    for c in range(sc_n):
        ot_ps = tp_psum.tile([P, P], f32, name="ot_ps", tag="tp")
        nc.tensor.matmul(
            ot_ps[:], mo_sb[:, c * P:(c + 1) * P], ident[:],
            is_transpose=True, start=True, stop=True,
        )
        oc_sb = fin_pool.tile([P, dout], f32, name="oc_sb")
        nc.scalar.activation(oc_sb[:], ot_ps[:], AF.Relu, scale=rrT_sb[:, c:c + 1])
        nc.sync.dma_start(out[c * P:(c + 1) * P, :], oc_sb[:])


def build_nc(n=N, s=S, din=DIN, dout=DOUT):
    from contextlib import ExitStack

    import concourse.bacc as bacc
    import concourse.tile as tile

    nc = bacc.Bacc(
        "TRN2",
        target_bir_lowering=False,
        debug=False,
        num_devices=NCORES,
    )
    with tile.TileContext(nc) as tc, ExitStack() as ctx:
        _emit(nc, tc, ctx, n, s, din, dout)
    nc.compile()
    return nc


def prep_adjt(adj_slab):
    """[s, n] adj row-slab -> transposed [n, s] bf16 with GP-row interleave."""
    import ml_dtypes

    adjt = adj_slab.T  # [n, s]
    n, s = adjt.shape
    P = 128
    g = n // (GP * P)
    adjt = adjt.reshape(g, GP, P, s).transpose(0, 2, 1, 3).reshape(n, s)
    return np.ascontiguousarray(adjt.astype(ml_dtypes.bfloat16))


def make_in_maps(x, adj, W, attn_self, attn_neigh, s=S):
    import ml_dtypes

    bf = ml_dtypes.bfloat16
    att = np.concatenate([attn_self, attn_neigh], axis=1).astype(bf)
    wmat = np.ascontiguousarray(W.astype(bf))
    wtt = np.ascontiguousarray(W.T.astype(bf))
    in_maps = []
    for c in range(NCORES):
        sl = slice(c * s, (c + 1) * s)
        in_maps.append({
            "adjt": prep_adjt(adj[sl, :]),
            "xt": np.ascontiguousarray(x[sl, :].T.astype(bf)),
            "wmat": wmat,
            "wt": wtt,
            "att": att,
        })
    return in_maps


def kernel(x, adj, W, attn_self, attn_neigh):
    from concourse.bass_utils import run_bass_kernel_spmd

    x = np.asarray(x, dtype=np.float32)
    adj = np.asarray(adj, dtype=np.float32)
    W = np.asarray(W, dtype=np.float32)
    attn_self = np.asarray(attn_self, dtype=np.float32)
    attn_neigh = np.asarray(attn_neigh, dtype=np.float32)

    nc = build_nc()
    in_maps = make_in_maps(x, adj, W, attn_self, attn_neigh)
    res = run_bass_kernel_spmd(nc, in_maps, list(range(NCORES)))
    return np.concatenate([res.results[c]["out"] for c in range(NCORES)], axis=0)


# revision 45
# speedup vs baseline: 1.0960x; 1.0960x over previous
"""AttentiveGraphConvolution (GAT-style layer) on 8 trn2 NeuronCores.

Math (reference):
    h   = x @ W                       [N, D]
    a_s = h @ attn_self               [N, 1]
    a_n = h @ attn_neigh              [N, 1]
    e   = leaky_relu(a_s + a_n.T, 0.2)
    e   = e + NEG_INF * (1 - adj)
    out = relu(softmax(e, -1) @ h)

Reformulation used here (exact in fp32 up to rounding):
    s_ij = a_s[i] + a_n[j]
    exp(leaky(s)) = exp(0.2 s) * max(exp(0.8 s), 1)       (leaky alpha = 0.2)
    exp(0.8 s)    = w[i] * w2[j],  w = e^{0.8 a_s}, w2 = e^{0.8 a_n}
    adj binary =>  masked weight t_ij = adj_ij * u2_i * v2_j * max(w_i w2_j, 1)

    out_i = relu( (sum_j t_ij h_j) / (sum_j t_ij) )
          = relu( (sum_j q_ji h2_j) / (sum_j q_ji v2_j) )   (u2_i cancels)
    with q_ji  = adjT_ji * max(w_i w2_j, 1)                 [j, i] layout
         h2_j  = v2_j * h_j

Per adj tile the device work is:  R = w2_j * W_bcast  (ACT copy-with-scale),
q = (R max 1) * adjT  (DVE scalar_tensor_tensor), then accumulating float32r
matmuls  outT += h2_chunk.T @ q  and  rs += v2_chunk.T @ q  on the PE.

Changes vs the first-pass kernel (profile-driven):
  * ONE collective instead of two.  Each NRT collective costs ~20 us of
    serial CC-stream time on top of the launch-stagger rendezvous, and the
    trace showed a further 33 us gpsimd stall between the a_n AllGather and
    the h2 AllGather trigger.  a_n now rides in the same gathered tensor as
    h2 (8 extra rows appended to the 1024-row partition-major h2 block).
  * a_s/a_n are computed straight from x via av2 = [W@attn_self|W@attn_neigh]
    (a [2,512] matmul then av = av2T @ x), so h2 production and the gather
    trigger no longer wait for the full hT pass.
  * x and W ship as bf16 (a_v/h accumulate in fp32 PSUM; rel-err stays ~5e-3,
    well inside the 2e-2 gate), halving the phase-1 DMA.
  * The gathered h2 lands in 8 per-source-core tiles so the first matmul
    only waits on block 0's read-back, not all eight.
The main accumulation loop is untouched from the validated baseline.

Sharding: output rows across 8 cores. Each core receives its adj row-slab as
bf16 (adj is binary so bf16 is exact), pre-transposed and row-interleaved in
groups of GP=4 (host layout choice) so each DMA descriptor covers 4 adjacency
rows = 8 KB contiguous.
"""

import numpy as np

N = 8192
DIN = 512
DOUT = 128
NCORES = 8
S = N // NCORES     # 1024 output rows per core
GP = 4              # adjacency rows per partition per DMA (descriptor size)


def _emit(nc, tc, ctx, n, s, din, dout):
    from concourse import masks, mybir

    f32 = mybir.dt.float32
    f32r = mybir.dt.float32r
    bf16 = mybir.dt.bfloat16
    AF = mybir.ActivationFunctionType
    ALU = mybir.AluOpType

    P = 128
    jc_n = n // P       # j chunks over all nodes
    sc_n = s // P       # chunks in the local row slab
    kc_n = din // P     # contraction chunks for x @ W
    nb = min(512, s)    # matmul moving-dim block
    ib_n = s // nb      # i blocks per core (free dim of main matmuls)
    g_n = jc_n // GP    # adj super-chunks (GP j-chunks per DMA)
    sr = s + sc_n       # gathered rows per core: 1024 h2 + 8 a_n rows

    adjt = nc.dram_tensor("adjt", [n, s], bf16, kind="ExternalInput")
    xt = nc.dram_tensor("xt", [din, s], bf16, kind="ExternalInput")
    wmat = nc.dram_tensor("wmat", [din, dout], bf16, kind="ExternalInput")
    wt = nc.dram_tensor("wt", [dout, din], bf16, kind="ExternalInput")
    att = nc.dram_tensor("att", [dout, 2], bf16, kind="ExternalInput")
    out = nc.dram_tensor("out", [s, dout], f32, kind="ExternalOutput")

    const_pool = ctx.enter_context(tc.tile_pool(name="const", bufs=1))
    ph1_pool = ctx.enter_context(tc.tile_pool(name="ph1", bufs=1))
    ph1_psum = ctx.enter_context(tc.tile_pool(name="ph1_psum", bufs=1, space="PSUM"))
    tp_psum = ctx.enter_context(tc.tile_pool(name="tp_psum", bufs=2, space="PSUM"))
    acc_psum = ctx.enter_context(tc.tile_pool(name="acc_psum", bufs=1, space="PSUM"))
    dram_pool = ctx.enter_context(tc.tile_pool(name="dram", bufs=1, space="DRAM"))
    adj_pool = ctx.enter_context(tc.tile_pool(name="adj", bufs=6))
    r_pool = ctx.enter_context(tc.tile_pool(name="r", bufs=5))
    q_pool = ctx.enter_context(tc.tile_pool(name="q", bufs=8))
    fin_pool = ctx.enter_context(tc.tile_pool(name="fin", bufs=2))

    ident = const_pool.tile([P, P], f32, name="ident")
    masks.make_identity(nc, ident[:])
    identb = const_pool.tile([P, P], bf16, name="identb")
    nc.scalar.activation(identb[:], ident[:], AF.Copy)
    identr = const_pool.tile([P, P], f32r, name="identr")
    nc.scalar.activation(identr[:], ident[:], AF.Copy)

    # ---- Phase 1: input DMAs, attention vectors straight from x ----------
    w_sb = []
    x_sb = []
    for k in range(kc_n):
        wk = ph1_pool.tile([P, dout], bf16, name="w_sb", tag=f"w_sb{k}")
        nc.sync.dma_start(wk[:], wmat[k * P:(k + 1) * P, :])
        w_sb.append(wk)
        xk = ph1_pool.tile([P, s], bf16, name="x_sb", tag=f"x_sb{k}")
        nc.sync.dma_start(xk[:], xt[k * P:(k + 1) * P, :])
        x_sb.append(xk)
    wt_sb = ph1_pool.tile([P, din], bf16, name="wt_sb")
    nc.sync.dma_start(wt_sb[:], wt[:])
    att_sb = const_pool.tile([P, 2], bf16, name="att_sb")
    nc.sync.dma_start(att_sb[:], att[:])

    # av2 = [W@attn_self | W@attn_neigh].T : [2, din]
    av2_ps = tp_psum.tile([2, din], f32, name="av2_ps", tag="tp")
    nc.tensor.matmul(av2_ps[:], att_sb[:], wt_sb[:], start=True, stop=True)
    av2_sb = ph1_pool.tile([2, din], bf16, name="av2_sb")
    nc.scalar.activation(av2_sb[:], av2_ps[:], AF.Copy)
    av2T_sb = []
    for k in range(kc_n):
        avT_ps = tp_psum.tile([P, 2], bf16, name="avT_ps", tag="tp")
        nc.tensor.matmul(
            avT_ps[:], av2_sb[:, k * P:(k + 1) * P], identb[:2, :2],
            is_transpose=True, start=True, stop=True,
        )
        a2t = ph1_pool.tile([P, 2], bf16, name="av2T_sb", tag=f"av2T{k}")
        nc.scalar.activation(a2t[:], avT_ps[:], AF.Copy)
        av2T_sb.append(a2t)
    # av[2, s] = [a_s ; a_n] for the local slab, straight from x
    av_sb = ph1_pool.tile([2, s], f32r, name="av_sb")
    for b in range(ib_n):
        avl_ps = tp_psum.tile([2, nb], f32, name="avl_ps", tag="tp")
        for k in range(kc_n):
            nc.tensor.matmul(
                avl_ps[:], av2T_sb[k][:], x_sb[k][:, b * nb:(b + 1) * nb],
                start=(k == 0), stop=(k == kc_n - 1),
            )
        nc.scalar.activation(av_sb[:, b * nb:(b + 1) * nb], avl_ps[:], AF.Copy)

    # W_bcast[p, i] = exp(0.8 * a_s_local[i]) for every partition p
    wrow_sb = ph1_pool.tile([1, s], bf16, name="wrow_sb")
    nc.scalar.activation(wrow_sb[:], av_sb[0:1, :], AF.Exp, scale=0.8)
    ones_sb = const_pool.tile([1, P], bf16, name="ones_sb")
    nc.gpsimd.memset(ones_sb[:], 1.0)
    wb_sb = const_pool.tile([P, s], bf16, name="wb_sb")
    for b in range(ib_n):
        wb_ps = tp_psum.tile([P, nb], f32, name="wb_ps", tag="tp")
        nc.tensor.matmul(
            wb_ps[:], ones_sb[:], wrow_sb[:, b * nb:(b + 1) * nb],
            start=True, stop=True,
        )
        nc.scalar.activation(wb_sb[:, b * nb:(b + 1) * nb], wb_ps[:], AF.Copy)

    # hT[d, n_local] = (x @ W).T for the local slab
    hT_sb = ph1_pool.tile([P, s], f32, name="hT_sb")
    for b in range(ib_n):
        hT_ps = ph1_psum.tile([P, nb], f32, name="hT_ps")
        for k in range(kc_n):
            nc.tensor.matmul(
                hT_ps[:],
                w_sb[k][:],
                x_sb[k][:, b * nb:(b + 1) * nb],
                start=(k == 0), stop=(k == kc_n - 1),
            )
        nc.scalar.activation(hT_sb[:, b * nb:(b + 1) * nb], hT_ps[:], AF.Copy)

    # ---- Phase 2: h2 shard + a_n packed into ONE gathered tensor ---------
    # Local chunk c is written to rows {p*sc_n + c} so the gathered tensor
    # reads back with 4 KB-contiguous per-partition descriptors; rows
    # s..s+sc_n-1 carry a_n for the local slab (row t = nodes t*128..).
    anT_sb = ph1_pool.tile([P, sc_n], f32, name="anT_sb")
    for c in range(sc_n):
        avT2_ps = tp_psum.tile([P, 2], f32r, name="avT2_ps", tag="tp")
        nc.tensor.matmul(
            avT2_ps[:], av_sb[:, c * P:(c + 1) * P], identr[:2, :2],
            is_transpose=True, start=True, stop=True,
        )
        nc.scalar.activation(anT_sb[:, c:c + 1], avT2_ps[:, 1:2], AF.Copy)
    eanloc_sb = ph1_pool.tile([P, sc_n], f32, name="eanloc_sb")
    nc.scalar.activation(eanloc_sb[:], anT_sb[:], AF.Exp, scale=1.0)

    h2an_dram = dram_pool.tile([sr, dout], bf16, name="h2an_dram")
    h2an_pm = h2an_dram[0:s].rearrange("(p kl) d -> kl p d", kl=sc_n)
    for c in range(sc_n):
        hn_ps = tp_psum.tile([P, P], f32, name="hn_ps", tag="tp")
        nc.tensor.matmul(
            hn_ps[:], hT_sb[:, c * P:(c + 1) * P], ident[:],
            is_transpose=True, start=True, stop=True,
        )
        h2c_sb = fin_pool.tile([P, dout], bf16, name="h2c_sb")
        nc.scalar.activation(h2c_sb[:], hn_ps[:], AF.Copy,
                             scale=eanloc_sb[:, c:c + 1])
        nc.sync.dma_start(h2an_pm[c], h2c_sb[:])
    # a_n rows [8, 128] bf16: transpose anT back to node order for the tail
    anTb_sb = ph1_pool.tile([P, sc_n], bf16, name="anTb_sb")
    nc.scalar.activation(anTb_sb[:], anT_sb[:], AF.Copy)
    anb_ps = tp_psum.tile([sc_n, P], bf16, name="anb_ps", tag="tp")
    nc.tensor.matmul(anb_ps[:], anTb_sb[:], identb[:],
                     is_transpose=True, start=True, stop=True)
    anrow_sb = ph1_pool.tile([sc_n, P], bf16, name="anrow_sb")
    nc.scalar.activation(anrow_sb[:], anb_ps[:], AF.Copy)
    nc.sync.dma_start(h2an_dram[s:sr], anrow_sb[:])

    groups = [list(range(NCORES))]
    h2full_dram = dram_pool.tile([NCORES * sr, dout], bf16, addr_space="Shared",
                                 name="h2full")
    nc.gpsimd.collective_compute(
        "AllGather", ALU.bypass, replica_groups=groups,
        ins=[h2an_dram.opt()], outs=[h2full_dram.opt()],
    )

    # ---- Phase 3: unpack gathered a_n -> w2/v2, h2 block tiles -----------
    anf_raw = ph1_pool.tile([jc_n, P], bf16, name="anf_raw")
    for cc in range(NCORES):
        nc.sync.dma_start(
            anf_raw[cc * sc_n:(cc + 1) * sc_n, :],
            h2full_dram[cc * sr + s:(cc + 1) * sr, :],
        )
    anf_ps = tp_psum.tile([P, jc_n], bf16, name="anf_ps", tag="tp")
    nc.tensor.matmul(anf_ps[:], anf_raw[:], identb[:jc_n, :jc_n],
                     is_transpose=True, start=True, stop=True)
    m_sb = const_pool.tile([P, jc_n], f32, name="m_sb")
    nc.scalar.activation(m_sb[:], anf_ps[:], AF.Exp, scale=-0.8)
    ean_sb = const_pool.tile([P, jc_n], bf16, name="ean_sb")
    nc.scalar.activation(ean_sb[:], anf_ps[:], AF.Exp, scale=1.0)

    # gathered h2 as 8 per-source-core tiles (first matmul waits on block 0
    # only); 4 KB per-partition descriptors via the partition-major layout
    h2blk = []
    for cc in range(NCORES):
        hb = ph1_pool.tile([P, sc_n * dout], bf16, name="h2blk", tag=f"h2b{cc}")
        nc.sync.dma_start(
            hb[:],
            h2full_dram[cc * sr:cc * sr + s, :].rearrange(
                "(p kl) d -> p (kl d)", kl=sc_n),
        )
        h2blk.append(hb)

    # adjacency stream (whole slab, ring of 6 super-chunks)
    adj_t = []
    for g in range(g_n):
        at = adj_pool.tile([P, GP * s], bf16, name="adj_t")
        nc.sync.dma_start(
            at[:],
            adjt[g * GP * P:(g + 1) * GP * P, :].rearrange(
                "(p r) i -> p (r i)", r=GP),
        )
        adj_t.append(at)

    # ---- Phase 4: main loop over adj super-chunks (unchanged) ------------
    mm_ps = [acc_psum.tile([P, nb], f32, name=f"mm_ps{b}") for b in range(ib_n)]
    rs_ps = [acc_psum.tile([1, nb], f32, name=f"rs_ps{b}") for b in range(ib_n)]
    for g in range(g_n):
        for r in range(GP):
            j = g * GP + r
            q_t = q_pool.tile([P, s], bf16, name="q_t")
            nc.vector.scalar_tensor_tensor(
                q_t[:], wb_sb[:], m_sb[:, j:j + 1],
                adj_t[g][:, r * s:(r + 1) * s],
                op0=ALU.max, op1=ALU.mult,
            )
            st = h2blk[j // sc_n][:, (j % sc_n) * dout:(j % sc_n + 1) * dout]
            for b in range(ib_n):
                nc.tensor.matmul(
                    mm_ps[b][:], st, q_t[:, b * nb:(b + 1) * nb],
                    start=(j == 0), stop=(j == jc_n - 1),
                )
            for b in range(ib_n):
                nc.tensor.matmul(
                    rs_ps[b][:], ean_sb[:, j:j + 1], q_t[:, b * nb:(b + 1) * nb],
                    start=(j == 0), stop=(j == jc_n - 1),
                )

    # ---- Phase 5: normalize, relu, transpose out -------------------------
    rs_sb = ph1_pool.tile([1, s], f32, name="rs_sb")
    for b in range(ib_n):
        nc.scalar.activation(rs_sb[:, b * nb:(b + 1) * nb], rs_ps[b][:], AF.Copy)
    rs_dram = dram_pool.tile([sc_n, P], f32, name="rs_dram")
    nc.sync.dma_start(rs_dram[:].rearrange("k p -> (k p)")[None, :], rs_sb[0:1, :])
    rs_raw = ph1_pool.tile([sc_n, P], f32, name="rs_raw")
    nc.sync.dma_start(rs_raw[:], rs_dram[:])
    rsT_ps = tp_psum.tile([P, sc_n], f32, name="rsT_ps", tag="tp")
    nc.tensor.matmul(rsT_ps[:], rs_raw[:], ident[:sc_n, :sc_n],
                     is_transpose=True, start=True, stop=True)
    rrT_sb = ph1_pool.tile([P, sc_n], f32, name="rrT_sb")
    nc.vector.reciprocal(rrT_sb[:], rsT_ps[:])

    mo_sb = ph1_pool.tile([P, s], f32, name="mo_sb")
    for b in range(ib_n):
        nc.scalar.activation(mo_sb[:, b * nb:(b + 1) * nb], mm_ps[b][:], AF.Copy)
    for c in range(sc_n):
        ot_ps = tp_psum.tile([P, P], f32, name="ot_ps", tag="tp")
        nc.tensor.matmul(
            ot_ps[:], mo_sb[:, c * P:(c + 1) * P], ident[:],
            is_transpose=True, start=True, stop=True,
        )
        oc_sb = fin_pool.tile([P, dout], f32, name="oc_sb")
        nc.scalar.activation(oc_sb[:], ot_ps[:], AF.Relu, scale=rrT_sb[:, c:c + 1])
        nc.sync.dma_start(out[c * P:(c + 1) * P, :], oc_sb[:])


def build_nc(n=N, s=S, din=DIN, dout=DOUT):
    from contextlib import ExitStack

    import concourse.bacc as bacc
    import concourse.tile as tile

    nc = bacc.Bacc(
        "TRN2",
        target_bir_lowering=False,
        debug=False,
        num_devices=NCORES,
    )
    with tile.TileContext(nc) as tc, ExitStack() as ctx:
        _emit(nc, tc, ctx, n, s, din, dout)
    nc.compile()
    return nc


def prep_adjt(adj_slab):
    """[s, n] adj row-slab -> transposed [n, s] bf16 with GP-row interleave."""
    import ml_dtypes

    adjt = adj_slab.T  # [n, s]
    n, s = adjt.shape
    P = 128
    g = n // (GP * P)
    adjt = adjt.reshape(g, GP, P, s).transpose(0, 2, 1, 3).reshape(n, s)
    return np.ascontiguousarray(adjt.astype(ml_dtypes.bfloat16))


def make_in_maps(x, adj, W, attn_self, attn_neigh, s=S):
    import ml_dtypes

    bf = ml_dtypes.bfloat16
    att = np.concatenate([attn_self, attn_neigh], axis=1).astype(bf)
    wmat = np.ascontiguousarray(W.astype(bf))
    wtt = np.ascontiguousarray(W.T.astype(bf))
    in_maps = []
    for c in range(NCORES):
        sl = slice(c * s, (c + 1) * s)
        in_maps.append({
            "adjt": prep_adjt(adj[sl, :]),
            "xt": np.ascontiguousarray(x[sl, :].T.astype(bf)),
            "wmat": wmat,
            "wt": wtt,
            "att": att,
        })
    return in_maps


def kernel(x, adj, W, attn_self, attn_neigh):
    from concourse.bass_utils import run_bass_kernel_spmd

    x = np.asarray(x, dtype=np.float32)
    adj = np.asarray(adj, dtype=np.float32)
    W = np.asarray(W, dtype=np.float32)
    attn_self = np.asarray(attn_self, dtype=np.float32)
    attn_neigh = np.asarray(attn_neigh, dtype=np.float32)

    nc = build_nc()
    in_maps = make_in_maps(x, adj, W, attn_self, attn_neigh)
    res = run_bass_kernel_spmd(nc, in_maps, list(range(NCORES)))
    return np.concatenate([res.results[c]["out"] for c in range(NCORES)], axis=0)


# revision 48
# speedup vs baseline: 1.1298x; 1.0309x over previous
"""AttentiveGraphConvolution (GAT-style layer) on 8 trn2 NeuronCores.

Math (reference):
    h   = x @ W                       [N, D]
    a_s = h @ attn_self               [N, 1]
    a_n = h @ attn_neigh              [N, 1]
    e   = leaky_relu(a_s + a_n.T, 0.2)
    e   = e + NEG_INF * (1 - adj)
    out = relu(softmax(e, -1) @ h)

Reformulation used here (exact in fp32 up to rounding):
    s_ij = a_s[i] + a_n[j]
    exp(leaky(s)) = exp(0.2 s) * max(exp(0.8 s), 1)       (leaky alpha = 0.2)
    exp(0.8 s)    = w[i] * w2[j],  w = e^{0.8 a_s}, w2 = e^{0.8 a_n}
    adj binary =>  masked weight t_ij = adj_ij * u2_i * v2_j * max(w_i w2_j, 1)

    out_i = relu( (sum_j t_ij h_j) / (sum_j t_ij) )
          = relu( (sum_j q_ji h2_j) / (sum_j q_ji v2_j) )   (u2_i cancels)
    with q_ji  = adjT_ji * max(w_i w2_j, 1)                 [j, i] layout
         h2_j  = v2_j * h_j

Per adj tile the device work is:  R = w2_j * W_bcast  (ACT copy-with-scale),
q = (R max 1) * adjT  (DVE scalar_tensor_tensor), then accumulating float32r
matmuls  outT += h2_chunk.T @ q  and  rs += v2_chunk.T @ q  on the PE.

Changes vs the first-pass kernel (profile-driven):
  * ONE collective instead of two.  Each NRT collective costs ~20 us of
    serial CC-stream time on top of the launch-stagger rendezvous, and the
    trace showed a further 33 us gpsimd stall between the a_n AllGather and
    the h2 AllGather trigger.  a_n now rides in the same gathered tensor as
    h2 (8 extra rows appended to the 1024-row partition-major h2 block).
  * a_s/a_n are computed straight from x via av2 = [W@attn_self|W@attn_neigh]
    (a [2,512] matmul then av = av2T @ x), so h2 production and the gather
    trigger no longer wait for the full hT pass.
  * x and W ship as bf16 (a_v/h accumulate in fp32 PSUM; rel-err stays ~5e-3,
    well inside the 2e-2 gate), halving the phase-1 DMA.
  * The gathered h2 lands in 8 per-source-core tiles so the first matmul
    only waits on block 0's read-back, not all eight.
The main accumulation loop is untouched from the validated baseline.

Sharding: output rows across 8 cores. Each core receives its adj row-slab as
bf16 (adj is binary so bf16 is exact), pre-transposed and row-interleaved in
groups of GP=4 (host layout choice) so each DMA descriptor covers 4 adjacency
rows = 8 KB contiguous.
"""

import numpy as np

N = 8192
DIN = 512
DOUT = 128
NCORES = 8
S = N // NCORES     # 1024 output rows per core
GP = 4              # adjacency rows per partition per DMA (descriptor size)


def _emit(nc, tc, ctx, n, s, din, dout):
    from concourse import masks, mybir

    f32 = mybir.dt.float32
    f32r = mybir.dt.float32r
    bf16 = mybir.dt.bfloat16
    AF = mybir.ActivationFunctionType
    ALU = mybir.AluOpType

    P = 128
    jc_n = n // P       # j chunks over all nodes
    sc_n = s // P       # chunks in the local row slab
    kc_n = din // P     # contraction chunks for x @ W
    nb = min(512, s)    # matmul moving-dim block
    ib_n = s // nb      # i blocks per core (free dim of main matmuls)
    g_n = jc_n // GP    # adj super-chunks (GP j-chunks per DMA)
    sr = s + sc_n       # gathered rows per core: 1024 h2 + 8 a_n rows

    adjt = nc.dram_tensor("adjt", [n, s], bf16, kind="ExternalInput")
    xt = nc.dram_tensor("xt", [din, s], bf16, kind="ExternalInput")
    wmat = nc.dram_tensor("wmat", [din, dout], bf16, kind="ExternalInput")
    wt = nc.dram_tensor("wt", [dout, din], bf16, kind="ExternalInput")
    att = nc.dram_tensor("att", [dout, 2], bf16, kind="ExternalInput")
    out = nc.dram_tensor("out", [s, dout], f32, kind="ExternalOutput")

    const_pool = ctx.enter_context(tc.tile_pool(name="const", bufs=1))
    ph1_pool = ctx.enter_context(tc.tile_pool(name="ph1", bufs=1))
    ph1_psum = ctx.enter_context(tc.tile_pool(name="ph1_psum", bufs=1, space="PSUM"))
    tp_psum = ctx.enter_context(tc.tile_pool(name="tp_psum", bufs=2, space="PSUM"))
    acc_psum = ctx.enter_context(tc.tile_pool(name="acc_psum", bufs=1, space="PSUM"))
    dram_pool = ctx.enter_context(tc.tile_pool(name="dram", bufs=1, space="DRAM"))
    adj_pool = ctx.enter_context(tc.tile_pool(name="adj", bufs=6))
    r_pool = ctx.enter_context(tc.tile_pool(name="r", bufs=5))
    q_pool = ctx.enter_context(tc.tile_pool(name="q", bufs=8))
    fin_pool = ctx.enter_context(tc.tile_pool(name="fin", bufs=2))

    ident = const_pool.tile([P, P], f32, name="ident")
    masks.make_identity(nc, ident[:])
    identb = const_pool.tile([P, P], bf16, name="identb")
    nc.scalar.activation(identb[:], ident[:], AF.Copy)
    identr = const_pool.tile([P, P], f32r, name="identr")
    nc.scalar.activation(identr[:], ident[:], AF.Copy)

    # ---- Phase 1: input DMAs, attention vectors straight from x ----------
    w_sb = []
    x_sb = []
    for k in range(kc_n):
        wk = ph1_pool.tile([P, dout], bf16, name="w_sb", tag=f"w_sb{k}")
        nc.sync.dma_start(wk[:], wmat[k * P:(k + 1) * P, :])
        w_sb.append(wk)
        xk = ph1_pool.tile([P, s], bf16, name="x_sb", tag=f"x_sb{k}")
        nc.sync.dma_start(xk[:], xt[k * P:(k + 1) * P, :])
        x_sb.append(xk)
    wt_sb = ph1_pool.tile([P, din], bf16, name="wt_sb")
    nc.sync.dma_start(wt_sb[:], wt[:])
    att_sb = const_pool.tile([P, 2], bf16, name="att_sb")
    nc.sync.dma_start(att_sb[:], att[:])

    # av2 = [W@attn_self | W@attn_neigh].T : [2, din]
    av2_ps = tp_psum.tile([2, din], f32, name="av2_ps", tag="tp")
    nc.tensor.matmul(av2_ps[:], att_sb[:], wt_sb[:], start=True, stop=True)
    av2_sb = ph1_pool.tile([2, din], bf16, name="av2_sb")
    nc.scalar.activation(av2_sb[:], av2_ps[:], AF.Copy)
    av2T_sb = []
    for k in range(kc_n):
        avT_ps = tp_psum.tile([P, 2], bf16, name="avT_ps", tag="tp")
        nc.tensor.matmul(
            avT_ps[:], av2_sb[:, k * P:(k + 1) * P], identb[:2, :2],
            is_transpose=True, start=True, stop=True,
        )
        a2t = ph1_pool.tile([P, 2], bf16, name="av2T_sb", tag=f"av2T{k}")
        nc.scalar.activation(a2t[:], avT_ps[:], AF.Copy)
        av2T_sb.append(a2t)
    # av[2, s] = [a_s ; a_n] for the local slab, straight from x
    av_sb = ph1_pool.tile([2, s], f32r, name="av_sb")
    for b in range(ib_n):
        avl_ps = tp_psum.tile([2, nb], f32, name="avl_ps", tag="tp")
        for k in range(kc_n):
            nc.tensor.matmul(
                avl_ps[:], av2T_sb[k][:], x_sb[k][:, b * nb:(b + 1) * nb],
                start=(k == 0), stop=(k == kc_n - 1),
            )
        nc.scalar.activation(av_sb[:, b * nb:(b + 1) * nb], avl_ps[:], AF.Copy)

    # W_bcast[p, i] = exp(0.8 * a_s_local[i]) for every partition p
    wrow_sb = ph1_pool.tile([1, s], bf16, name="wrow_sb")
    nc.scalar.activation(wrow_sb[:], av_sb[0:1, :], AF.Exp, scale=0.8)
    ones_sb = const_pool.tile([1, P], bf16, name="ones_sb")
    nc.gpsimd.memset(ones_sb[:], 1.0)
    wb_sb = const_pool.tile([P, s], bf16, name="wb_sb")
    for b in range(ib_n):
        wb_ps = tp_psum.tile([P, nb], f32, name="wb_ps", tag="tp")
        nc.tensor.matmul(
            wb_ps[:], ones_sb[:], wrow_sb[:, b * nb:(b + 1) * nb],
            start=True, stop=True,
        )
        nc.scalar.activation(wb_sb[:, b * nb:(b + 1) * nb], wb_ps[:], AF.Copy)

    # hT[d, n_local] = (x @ W).T for the local slab
    hT_sb = ph1_pool.tile([P, s], f32, name="hT_sb")
    for b in range(ib_n):
        hT_ps = ph1_psum.tile([P, nb], f32, name="hT_ps")
        for k in range(kc_n):
            nc.tensor.matmul(
                hT_ps[:],
                w_sb[k][:],
                x_sb[k][:, b * nb:(b + 1) * nb],
                start=(k == 0), stop=(k == kc_n - 1),
            )
        nc.scalar.activation(hT_sb[:, b * nb:(b + 1) * nb], hT_ps[:], AF.Copy)

    # ---- Phase 2: h2 shard + a_n packed into ONE gathered tensor ---------
    # Local chunk c is written to rows {p*sc_n + c} so the gathered tensor
    # reads back with 4 KB-contiguous per-partition descriptors; rows
    # s..s+sc_n-1 carry a_n for the local slab (row t = nodes t*128..).
    anT_sb = ph1_pool.tile([P, sc_n], f32, name="anT_sb")
    for c in range(sc_n):
        avT2_ps = tp_psum.tile([P, 2], f32r, name="avT2_ps", tag="tp")
        nc.tensor.matmul(
            avT2_ps[:], av_sb[:, c * P:(c + 1) * P], identr[:2, :2],
            is_transpose=True, start=True, stop=True,
        )
        nc.scalar.activation(anT_sb[:, c:c + 1], avT2_ps[:, 1:2], AF.Copy)
    eanloc_sb = ph1_pool.tile([P, sc_n], f32, name="eanloc_sb")
    nc.scalar.activation(eanloc_sb[:], anT_sb[:], AF.Exp, scale=1.0)

    h2an_dram = dram_pool.tile([sr, dout], bf16, name="h2an_dram")
    h2an_pm = h2an_dram[0:s].rearrange("(p kl) d -> kl p d", kl=sc_n)
    for c in range(sc_n):
        hn_ps = tp_psum.tile([P, P], f32, name="hn_ps", tag="tp")
        nc.tensor.matmul(
            hn_ps[:], hT_sb[:, c * P:(c + 1) * P], ident[:],
            is_transpose=True, start=True, stop=True,
        )
        h2c_sb = fin_pool.tile([P, dout], bf16, name="h2c_sb")
        nc.scalar.activation(h2c_sb[:], hn_ps[:], AF.Copy,
                             scale=eanloc_sb[:, c:c + 1])
        nc.sync.dma_start(h2an_pm[c], h2c_sb[:])
    # a_n rows [8, 128] bf16: transpose anT back to node order for the tail
    anTb_sb = ph1_pool.tile([P, sc_n], bf16, name="anTb_sb")
    nc.scalar.activation(anTb_sb[:], anT_sb[:], AF.Copy)
    anb_ps = tp_psum.tile([sc_n, P], bf16, name="anb_ps", tag="tp")
    nc.tensor.matmul(anb_ps[:], anTb_sb[:], identb[:],
                     is_transpose=True, start=True, stop=True)
    anrow_sb = ph1_pool.tile([sc_n, P], bf16, name="anrow_sb")
    nc.scalar.activation(anrow_sb[:], anb_ps[:], AF.Copy)
    nc.sync.dma_start(h2an_dram[s:sr], anrow_sb[:])

    groups = [list(range(NCORES))]
    h2full_dram = dram_pool.tile([NCORES * sr, dout], bf16, addr_space="Shared",
                                 name="h2full")
    nc.gpsimd.collective_compute(
        "AllGather", ALU.bypass, replica_groups=groups,
        ins=[h2an_dram.opt()], outs=[h2full_dram.opt()],
    )

    # ---- Phase 3: unpack gathered a_n -> w2/v2, h2 block tiles -----------
    anf_raw = ph1_pool.tile([jc_n, P], bf16, name="anf_raw")
    for cc in range(NCORES):
        nc.sync.dma_start(
            anf_raw[cc * sc_n:(cc + 1) * sc_n, :],
            h2full_dram[cc * sr + s:(cc + 1) * sr, :],
        )
    anf_ps = tp_psum.tile([P, jc_n], bf16, name="anf_ps", tag="tp")
    nc.tensor.matmul(anf_ps[:], anf_raw[:], identb[:jc_n, :jc_n],
                     is_transpose=True, start=True, stop=True)
    m_sb = const_pool.tile([P, jc_n], f32, name="m_sb")
    nc.scalar.activation(m_sb[:], anf_ps[:], AF.Exp, scale=-0.8)
    ean_sb = const_pool.tile([P, jc_n], bf16, name="ean_sb")
    nc.scalar.activation(ean_sb[:], anf_ps[:], AF.Exp, scale=1.0)

    # gathered h2 as 8 per-source-core tiles (first matmul waits on block 0
    # only); 4 KB per-partition descriptors via the partition-major layout
    h2blk = []
    for cc in range(NCORES):
        hb = ph1_pool.tile([P, sc_n * dout], bf16, name="h2blk", tag=f"h2b{cc}")
        nc.sync.dma_start(
            hb[:],
            h2full_dram[cc * sr:cc * sr + s, :].rearrange(
                "(p kl) d -> p (kl d)", kl=sc_n),
        )
        h2blk.append(hb)

    # adjacency stream (whole slab, ring of 6 super-chunks)
    adj_t = []
    for g in range(g_n):
        at = adj_pool.tile([P, GP * s], bf16, name="adj_t")
        nc.sync.dma_start(
            at[:],
            adjt[g * GP * P:(g + 1) * GP * P, :].rearrange(
                "(p r) i -> p (r i)", r=GP),
        )
        adj_t.append(at)

    # ---- Phase 4: main loop over adj super-chunks (unchanged) ------------
    mm_ps = [acc_psum.tile([P, nb], f32, name=f"mm_ps{b}") for b in range(ib_n)]
    rs_ps = [acc_psum.tile([1, nb], f32, name=f"rs_ps{b}") for b in range(ib_n)]
    for g in range(g_n):
        for r in range(GP):
            j = g * GP + r
            q_t = q_pool.tile([P, s], bf16, name="q_t")
            nc.vector.scalar_tensor_tensor(
                q_t[:], wb_sb[:], m_sb[:, j:j + 1],
                adj_t[g][:, r * s:(r + 1) * s],
                op0=ALU.max, op1=ALU.mult,
            )
            st = h2blk[j // sc_n][:, (j % sc_n) * dout:(j % sc_n + 1) * dout]
            for b in range(ib_n):
                nc.tensor.matmul(
                    mm_ps[b][:], st, q_t[:, b * nb:(b + 1) * nb],
                    start=(j == 0), stop=(j == jc_n - 1),
                )
            for b in range(ib_n):
                nc.tensor.matmul(
                    rs_ps[b][:], ean_sb[:, j:j + 1], q_t[:, b * nb:(b + 1) * nb],
                    start=(j == 0), stop=(j == jc_n - 1),
                )

    # ---- Phase 5: normalize, relu, transpose out -------------------------
    rs_sb = ph1_pool.tile([1, s], f32, name="rs_sb")
    for b in range(ib_n):
        nc.scalar.activation(rs_sb[:, b * nb:(b + 1) * nb], rs_ps[b][:], AF.Copy)
    rs_dram = dram_pool.tile([sc_n, P], f32, name="rs_dram")
    nc.sync.dma_start(rs_dram[:].rearrange("k p -> (k p)")[None, :], rs_sb[0:1, :])
    rs_raw = ph1_pool.tile([sc_n, P], f32, name="rs_raw")
    nc.sync.dma_start(rs_raw[:], rs_dram[:])
    rsT_ps = tp_psum.tile([P, sc_n], f32, name="rsT_ps", tag="tp")
    nc.tensor.matmul(rsT_ps[:], rs_raw[:], ident[:sc_n, :sc_n],
                     is_transpose=True, start=True, stop=True)
    rrT_sb = ph1_pool.tile([P, sc_n], f32, name="rrT_sb")
    nc.vector.reciprocal(rrT_sb[:], rsT_ps[:])

    mo_sb = ph1_pool.tile([P, s], f32, name="mo_sb")
    for b in range(ib_n):
        nc.scalar.activation(mo_sb[:, b * nb:(b + 1) * nb], mm_ps[b][:], AF.Copy)
    for c in range(sc_n):
        ot_ps = tp_psum.tile([P, P], f32, name="ot_ps", tag="tp")
        nc.tensor.matmul(
            ot_ps[:], mo_sb[:, c * P:(c + 1) * P], ident[:],
            is_transpose=True, start=True, stop=True,
        )
        oc_sb = fin_pool.tile([P, dout], f32, name="oc_sb")
        nc.scalar.activation(oc_sb[:], ot_ps[:], AF.Relu, scale=rrT_sb[:, c:c + 1])
        nc.sync.dma_start(out[c * P:(c + 1) * P, :], oc_sb[:])


def build_nc(n=N, s=S, din=DIN, dout=DOUT):
    from contextlib import ExitStack

    import concourse.bacc as bacc
    import concourse.tile as tile

    nc = bacc.Bacc(
        "TRN2",
        target_bir_lowering=False,
        debug=False,
        num_devices=NCORES,
    )
    with tile.TileContext(nc) as tc, ExitStack() as ctx:
        _emit(nc, tc, ctx, n, s, din, dout)
    nc.compile()
    return nc


def prep_adjt(adj_slab):
    """[s, n] adj row-slab -> transposed [n, s] bf16 with GP-row interleave."""
    import ml_dtypes

    adjt = adj_slab.T  # [n, s]
    n, s = adjt.shape
    P = 128
    g = n // (GP * P)
    adjt = adjt.reshape(g, GP, P, s).transpose(0, 2, 1, 3).reshape(n, s)
    return np.ascontiguousarray(adjt.astype(ml_dtypes.bfloat16))


def make_in_maps(x, adj, W, attn_self, attn_neigh, s=S):
    import ml_dtypes

    bf = ml_dtypes.bfloat16
    att = np.concatenate([attn_self, attn_neigh], axis=1).astype(bf)
    wmat = np.ascontiguousarray(W.astype(bf))
    wtt = np.ascontiguousarray(W.T.astype(bf))
    in_maps = []
    for c in range(NCORES):
        sl = slice(c * s, (c + 1) * s)
        in_maps.append({
            "adjt": prep_adjt(adj[sl, :]),
            "xt": np.ascontiguousarray(x[sl, :].T.astype(bf)),
            "wmat": wmat,
            "wt": wtt,
            "att": att,
        })
    return in_maps


def kernel(x, adj, W, attn_self, attn_neigh):
    from concourse.bass_utils import run_bass_kernel_spmd

    x = np.asarray(x, dtype=np.float32)
    adj = np.asarray(adj, dtype=np.float32)
    W = np.asarray(W, dtype=np.float32)
    attn_self = np.asarray(attn_self, dtype=np.float32)
    attn_neigh = np.asarray(attn_neigh, dtype=np.float32)

    nc = build_nc()
    in_maps = make_in_maps(x, adj, W, attn_self, attn_neigh)
    res = run_bass_kernel_spmd(nc, in_maps, list(range(NCORES)))
    return np.concatenate([res.results[c]["out"] for c in range(NCORES)], axis=0)


# revision 49
# speedup vs baseline: 1.1681x; 1.0339x over previous
"""AttentiveGraphConvolution (GAT-style layer) on 8 trn2 NeuronCores.

Math (reference):
    h   = x @ W                       [N, D]
    a_s = h @ attn_self               [N, 1]
    a_n = h @ attn_neigh              [N, 1]
    e   = leaky_relu(a_s + a_n.T, 0.2)
    e   = e + NEG_INF * (1 - adj)
    out = relu(softmax(e, -1) @ h)

Reformulation used here (exact in fp32 up to rounding):
    s_ij = a_s[i] + a_n[j]
    exp(leaky(s)) = exp(0.2 s) * max(exp(0.8 s), 1)       (leaky alpha = 0.2)
    exp(0.8 s)    = w[i] * w2[j],  w = e^{0.8 a_s}, w2 = e^{0.8 a_n}
    adj binary =>  masked weight t_ij = adj_ij * u2_i * v2_j * max(w_i w2_j, 1)

    out_i = relu( (sum_j t_ij h_j) / (sum_j t_ij) )
          = relu( (sum_j q_ji h2_j) / (sum_j q_ji v2_j) )   (u2_i cancels)
    with q_ji  = adjT_ji * max(w_i w2_j, 1)                 [j, i] layout
         h2_j  = v2_j * h_j

Per adj tile the device work is:  R = w2_j * W_bcast  (ACT copy-with-scale),
q = (R max 1) * adjT  (DVE scalar_tensor_tensor), then accumulating float32r
matmuls  outT += h2_chunk.T @ q  and  rs += v2_chunk.T @ q  on the PE.

Changes vs the first-pass kernel (profile-driven):
  * ONE collective instead of two.  Each NRT collective costs ~20 us of
    serial CC-stream time on top of the launch-stagger rendezvous, and the
    trace showed a further 33 us gpsimd stall between the a_n AllGather and
    the h2 AllGather trigger.  a_n now rides in the same gathered tensor as
    h2 (8 extra rows appended to the 1024-row partition-major h2 block).
  * a_s/a_n are computed straight from x via av2 = [W@attn_self|W@attn_neigh]
    (a [2,512] matmul then av = av2T @ x), so h2 production and the gather
    trigger no longer wait for the full hT pass.
  * x and W ship as bf16 (a_v/h accumulate in fp32 PSUM; rel-err stays ~5e-3,
    well inside the 2e-2 gate), halving the phase-1 DMA.
  * The gathered h2 lands in 8 per-source-core tiles so the first matmul
    only waits on block 0's read-back, not all eight.
The main accumulation loop is untouched from the validated baseline.

Sharding: output rows across 8 cores. Each core receives its adj row-slab as
bf16 (adj is binary so bf16 is exact), pre-transposed and row-interleaved in
groups of GP=4 (host layout choice) so each DMA descriptor covers 4 adjacency
rows = 8 KB contiguous.
"""

import numpy as np

N = 8192
DIN = 512
DOUT = 128
NCORES = 8
S = N // NCORES     # 1024 output rows per core
GP = 4              # adjacency rows per partition per DMA (descriptor size)


def _emit(nc, tc, ctx, n, s, din, dout):
    from concourse import masks, mybir

    f32 = mybir.dt.float32
    f32r = mybir.dt.float32r
    bf16 = mybir.dt.bfloat16
    AF = mybir.ActivationFunctionType
    ALU = mybir.AluOpType

    P = 128
    jc_n = n // P       # j chunks over all nodes
    sc_n = s // P       # chunks in the local row slab
    kc_n = din // P     # contraction chunks for x @ W
    nb = min(512, s)    # matmul moving-dim block
    ib_n = s // nb      # i blocks per core (free dim of main matmuls)
    g_n = jc_n // GP    # adj super-chunks (GP j-chunks per DMA)
    sr = s + sc_n       # gathered rows per core: 1024 h2 + 8 a_n rows

    adjt = nc.dram_tensor("adjt", [n, s], bf16, kind="ExternalInput")
    xt = nc.dram_tensor("xt", [din, s], bf16, kind="ExternalInput")
    wmat = nc.dram_tensor("wmat", [din, dout], bf16, kind="ExternalInput")
    wt = nc.dram_tensor("wt", [dout, din], bf16, kind="ExternalInput")
    att = nc.dram_tensor("att", [dout, 2], bf16, kind="ExternalInput")
    out = nc.dram_tensor("out", [s, dout], f32, kind="ExternalOutput")

    const_pool = ctx.enter_context(tc.tile_pool(name="const", bufs=1))
    ph1_pool = ctx.enter_context(tc.tile_pool(name="ph1", bufs=1))
    ph1_psum = ctx.enter_context(tc.tile_pool(name="ph1_psum", bufs=1, space="PSUM"))
    tp_psum = ctx.enter_context(tc.tile_pool(name="tp_psum", bufs=2, space="PSUM"))
    acc_psum = ctx.enter_context(tc.tile_pool(name="acc_psum", bufs=1, space="PSUM"))
    dram_pool = ctx.enter_context(tc.tile_pool(name="dram", bufs=1, space="DRAM"))
    adj_pool = ctx.enter_context(tc.tile_pool(name="adj", bufs=16))
    r_pool = ctx.enter_context(tc.tile_pool(name="r", bufs=5))
    q_pool = ctx.enter_context(tc.tile_pool(name="q", bufs=8))
    fin_pool = ctx.enter_context(tc.tile_pool(name="fin", bufs=2))

    ident = const_pool.tile([P, P], f32, name="ident")
    masks.make_identity(nc, ident[:])
    identb = const_pool.tile([P, P], bf16, name="identb")
    nc.scalar.activation(identb[:], ident[:], AF.Copy)
    identr = const_pool.tile([P, P], f32r, name="identr")
    nc.scalar.activation(identr[:], ident[:], AF.Copy)

    # ---- Phase 1: input DMAs, attention vectors straight from x ----------
    w_sb = []
    x_sb = []
    for k in range(kc_n):
        wk = ph1_pool.tile([P, dout], bf16, name="w_sb", tag=f"w_sb{k}")
        nc.sync.dma_start(wk[:], wmat[k * P:(k + 1) * P, :])
        w_sb.append(wk)
        xk = ph1_pool.tile([P, s], bf16, name="x_sb", tag=f"x_sb{k}")
        nc.sync.dma_start(xk[:], xt[k * P:(k + 1) * P, :])
        x_sb.append(xk)
    wt_sb = ph1_pool.tile([P, din], bf16, name="wt_sb")
    nc.sync.dma_start(wt_sb[:], wt[:])
    att_sb = const_pool.tile([P, 2], bf16, name="att_sb")
    nc.sync.dma_start(att_sb[:], att[:])

    # av2 = [W@attn_self | W@attn_neigh].T : [2, din]
    av2_ps = tp_psum.tile([2, din], f32, name="av2_ps", tag="tp")
    nc.tensor.matmul(av2_ps[:], att_sb[:], wt_sb[:], start=True, stop=True)
    av2_sb = ph1_pool.tile([2, din], bf16, name="av2_sb")
    nc.scalar.activation(av2_sb[:], av2_ps[:], AF.Copy)
    av2T_sb = []
    for k in range(kc_n):
        avT_ps = tp_psum.tile([P, 2], bf16, name="avT_ps", tag="tp")
        nc.tensor.matmul(
            avT_ps[:], av2_sb[:, k * P:(k + 1) * P], identb[:2, :2],
            is_transpose=True, start=True, stop=True,
        )
        a2t = ph1_pool.tile([P, 2], bf16, name="av2T_sb", tag=f"av2T{k}")
        nc.scalar.activation(a2t[:], avT_ps[:], AF.Copy)
        av2T_sb.append(a2t)
    # av[2, s] = [a_s ; a_n] for the local slab, straight from x
    av_sb = ph1_pool.tile([2, s], f32r, name="av_sb")
    for b in range(ib_n):
        avl_ps = tp_psum.tile([2, nb], f32, name="avl_ps", tag="tp")
        for k in range(kc_n):
            nc.tensor.matmul(
                avl_ps[:], av2T_sb[k][:], x_sb[k][:, b * nb:(b + 1) * nb],
                start=(k == 0), stop=(k == kc_n - 1),
            )
        nc.scalar.activation(av_sb[:, b * nb:(b + 1) * nb], avl_ps[:], AF.Copy)

    # W_bcast[p, i] = exp(0.8 * a_s_local[i]) for every partition p
    wrow_sb = ph1_pool.tile([1, s], bf16, name="wrow_sb")
    nc.scalar.activation(wrow_sb[:], av_sb[0:1, :], AF.Exp, scale=0.8)
    ones_sb = const_pool.tile([1, P], bf16, name="ones_sb")
    nc.gpsimd.memset(ones_sb[:], 1.0)
    wb_sb = const_pool.tile([P, s], bf16, name="wb_sb")
    for b in range(ib_n):
        wb_ps = tp_psum.tile([P, nb], f32, name="wb_ps", tag="tp")
        nc.tensor.matmul(
            wb_ps[:], ones_sb[:], wrow_sb[:, b * nb:(b + 1) * nb],
            start=True, stop=True,
        )
        nc.scalar.activation(wb_sb[:, b * nb:(b + 1) * nb], wb_ps[:], AF.Copy)

    # hT[d, n_local] = (x @ W).T for the local slab
    hT_sb = ph1_pool.tile([P, s], f32, name="hT_sb")
    for b in range(ib_n):
        hT_ps = ph1_psum.tile([P, nb], f32, name="hT_ps")
        for k in range(kc_n):
            nc.tensor.matmul(
                hT_ps[:],
                w_sb[k][:],
                x_sb[k][:, b * nb:(b + 1) * nb],
                start=(k == 0), stop=(k == kc_n - 1),
            )
        nc.scalar.activation(hT_sb[:, b * nb:(b + 1) * nb], hT_ps[:], AF.Copy)

    # ---- Phase 2: h2 shard + a_n packed into ONE gathered tensor ---------
    # Local chunk c is written to rows {p*sc_n + c} so the gathered tensor
    # reads back with 4 KB-contiguous per-partition descriptors; rows
    # s..s+sc_n-1 carry a_n for the local slab (row t = nodes t*128..).
    anT_sb = ph1_pool.tile([P, sc_n], f32, name="anT_sb")
    for c in range(sc_n):
        avT2_ps = tp_psum.tile([P, 2], f32r, name="avT2_ps", tag="tp")
        nc.tensor.matmul(
            avT2_ps[:], av_sb[:, c * P:(c + 1) * P], identr[:2, :2],
            is_transpose=True, start=True, stop=True,
        )
        nc.scalar.activation(anT_sb[:, c:c + 1], avT2_ps[:, 1:2], AF.Copy)
    eanloc_sb = ph1_pool.tile([P, sc_n], f32, name="eanloc_sb")
    nc.scalar.activation(eanloc_sb[:], anT_sb[:], AF.Exp, scale=1.0)

    h2an_dram = dram_pool.tile([sr, dout], bf16, name="h2an_dram")
    h2an_pm = h2an_dram[0:s].rearrange("(p kl) d -> kl p d", kl=sc_n)
    for c in range(sc_n):
        hn_ps = tp_psum.tile([P, P], f32, name="hn_ps", tag="tp")
        nc.tensor.matmul(
            hn_ps[:], hT_sb[:, c * P:(c + 1) * P], ident[:],
            is_transpose=True, start=True, stop=True,
        )
        h2c_sb = fin_pool.tile([P, dout], bf16, name="h2c_sb")
        nc.scalar.activation(h2c_sb[:], hn_ps[:], AF.Copy,
                             scale=eanloc_sb[:, c:c + 1])
        nc.sync.dma_start(h2an_pm[c], h2c_sb[:])
    # a_n rows [8, 128] bf16: transpose anT back to node order for the tail
    anTb_sb = ph1_pool.tile([P, sc_n], bf16, name="anTb_sb")
    nc.scalar.activation(anTb_sb[:], anT_sb[:], AF.Copy)
    anb_ps = tp_psum.tile([sc_n, P], bf16, name="anb_ps", tag="tp")
    nc.tensor.matmul(anb_ps[:], anTb_sb[:], identb[:],
                     is_transpose=True, start=True, stop=True)
    anrow_sb = ph1_pool.tile([sc_n, P], bf16, name="anrow_sb")
    nc.scalar.activation(anrow_sb[:], anb_ps[:], AF.Copy)
    nc.sync.dma_start(h2an_dram[s:sr], anrow_sb[:])

    groups = [list(range(NCORES))]
    h2full_dram = dram_pool.tile([NCORES * sr, dout], bf16, addr_space="Shared",
                                 name="h2full")
    nc.gpsimd.collective_compute(
        "AllGather", ALU.bypass, replica_groups=groups,
        ins=[h2an_dram.opt()], outs=[h2full_dram.opt()],
    )

    # ---- Phase 3: unpack gathered a_n -> w2/v2, h2 block tiles -----------
    anf_raw = ph1_pool.tile([jc_n, P], bf16, name="anf_raw")
    for cc in range(NCORES):
        nc.sync.dma_start(
            anf_raw[cc * sc_n:(cc + 1) * sc_n, :],
            h2full_dram[cc * sr + s:(cc + 1) * sr, :],
        )
    anf_ps = tp_psum.tile([P, jc_n], bf16, name="anf_ps", tag="tp")
    nc.tensor.matmul(anf_ps[:], anf_raw[:], identb[:jc_n, :jc_n],
                     is_transpose=True, start=True, stop=True)
    m_sb = const_pool.tile([P, jc_n], f32, name="m_sb")
    nc.scalar.activation(m_sb[:], anf_ps[:], AF.Exp, scale=-0.8)
    ean_sb = const_pool.tile([P, jc_n], bf16, name="ean_sb")
    nc.scalar.activation(ean_sb[:], anf_ps[:], AF.Exp, scale=1.0)

    # gathered h2 as 8 per-source-core tiles (first matmul waits on block 0
    # only); 4 KB per-partition descriptors via the partition-major layout
    h2blk = []
    for cc in range(NCORES):
        hb = ph1_pool.tile([P, sc_n * dout], bf16, name="h2blk", tag=f"h2b{cc}")
        nc.sync.dma_start(
            hb[:],
            h2full_dram[cc * sr:cc * sr + s, :].rearrange(
                "(p kl) d -> p (kl d)", kl=sc_n),
        )
        h2blk.append(hb)

    # adjacency stream (whole slab, ring of 6 super-chunks)
    adj_t = []
    for g in range(g_n):
        at = adj_pool.tile([P, GP * s], bf16, name="adj_t")
        nc.sync.dma_start(
            at[:],
            adjt[g * GP * P:(g + 1) * GP * P, :].rearrange(
                "(p r) i -> p (r i)", r=GP),
        )
        adj_t.append(at)

    # ---- Phase 4: main loop over adj super-chunks (unchanged) ------------
    mm_ps = [acc_psum.tile([P, nb], f32, name=f"mm_ps{b}") for b in range(ib_n)]
    rs_ps = [acc_psum.tile([1, nb], f32, name=f"rs_ps{b}") for b in range(ib_n)]
    for g in range(g_n):
        for r in range(GP):
            j = g * GP + r
            q_t = q_pool.tile([P, s], bf16, name="q_t")
            nc.vector.scalar_tensor_tensor(
                q_t[:], wb_sb[:], m_sb[:, j:j + 1],
                adj_t[g][:, r * s:(r + 1) * s],
                op0=ALU.max, op1=ALU.mult,
            )
            st = h2blk[j // sc_n][:, (j % sc_n) * dout:(j % sc_n + 1) * dout]
            for b in range(ib_n):
                nc.tensor.matmul(
                    mm_ps[b][:], st, q_t[:, b * nb:(b + 1) * nb],
                    start=(j == 0), stop=(j == jc_n - 1),
                )
            for b in range(ib_n):
                nc.tensor.matmul(
                    rs_ps[b][:], ean_sb[:, j:j + 1], q_t[:, b * nb:(b + 1) * nb],
                    start=(j == 0), stop=(j == jc_n - 1),
                )

    # ---- Phase 5: normalize, relu, transpose out -------------------------
    rs_sb = ph1_pool.tile([1, s], f32, name="rs_sb")
    for b in range(ib_n):
        nc.scalar.activation(rs_sb[:, b * nb:(b + 1) * nb], rs_ps[b][:], AF.Copy)
    rs_dram = dram_pool.tile([sc_n, P], f32, name="rs_dram")
    nc.sync.dma_start(rs_dram[:].rearrange("k p -> (k p)")[None, :], rs_sb[0:1, :])
    rs_raw = ph1_pool.tile([sc_n, P], f32, name="rs_raw")
    nc.sync.dma_start(rs_raw[:], rs_dram[:])
    rsT_ps = tp_psum.tile([P, sc_n], f32, name="rsT_ps", tag="tp")
    nc.tensor.matmul(rsT_ps[:], rs_raw[:], ident[:sc_n, :sc_n],
                     is_transpose=True, start=True, stop=True)
    rrT_sb = ph1_pool.tile([P, sc_n], f32, name="rrT_sb")
    nc.vector.reciprocal(rrT_sb[:], rsT_ps[:])

    mo_sb = ph1_pool.tile([P, s], f32, name="mo_sb")
    for b in range(ib_n):
        nc.scalar.activation(mo_sb[:, b * nb:(b + 1) * nb], mm_ps[b][:], AF.Copy)
    for c in range(sc_n):
        ot_ps = tp_psum.tile([P, P], f32, name="ot_ps", tag="tp")
        nc.tensor.matmul(
            ot_ps[:], mo_sb[:, c * P:(c + 1) * P], ident[:],
            is_transpose=True, start=True, stop=True,
        )
        oc_sb = fin_pool.tile([P, dout], f32, name="oc_sb")
        nc.scalar.activation(oc_sb[:], ot_ps[:], AF.Relu, scale=rrT_sb[:, c:c + 1])
        nc.sync.dma_start(out[c * P:(c + 1) * P, :], oc_sb[:])


def build_nc(n=N, s=S, din=DIN, dout=DOUT):
    from contextlib import ExitStack

    import concourse.bacc as bacc
    import concourse.tile as tile

    nc = bacc.Bacc(
        "TRN2",
        target_bir_lowering=False,
        debug=False,
        num_devices=NCORES,
    )
    with tile.TileContext(nc) as tc, ExitStack() as ctx:
        _emit(nc, tc, ctx, n, s, din, dout)
    nc.compile()
    return nc


def prep_adjt(adj_slab):
    """[s, n] adj row-slab -> transposed [n, s] bf16 with GP-row interleave."""
    import ml_dtypes

    adjt = adj_slab.T  # [n, s]
    n, s = adjt.shape
    P = 128
    g = n // (GP * P)
    adjt = adjt.reshape(g, GP, P, s).transpose(0, 2, 1, 3).reshape(n, s)
    return np.ascontiguousarray(adjt.astype(ml_dtypes.bfloat16))


def make_in_maps(x, adj, W, attn_self, attn_neigh, s=S):
    import ml_dtypes

    bf = ml_dtypes.bfloat16
    att = np.concatenate([attn_self, attn_neigh], axis=1).astype(bf)
    wmat = np.ascontiguousarray(W.astype(bf))
    wtt = np.ascontiguousarray(W.T.astype(bf))
    in_maps = []
    for c in range(NCORES):
        sl = slice(c * s, (c + 1) * s)
        in_maps.append({
            "adjt": prep_adjt(adj[sl, :]),
            "xt": np.ascontiguousarray(x[sl, :].T.astype(bf)),
            "wmat": wmat,
            "wt": wtt,
            "att": att,
        })
    return in_maps


def kernel(x, adj, W, attn_self, attn_neigh):
    from concourse.bass_utils import run_bass_kernel_spmd

    x = np.asarray(x, dtype=np.float32)
    adj = np.asarray(adj, dtype=np.float32)
    W = np.asarray(W, dtype=np.float32)
    attn_self = np.asarray(attn_self, dtype=np.float32)
    attn_neigh = np.asarray(attn_neigh, dtype=np.float32)

    nc = build_nc()
    in_maps = make_in_maps(x, adj, W, attn_self, attn_neigh)
    res = run_bass_kernel_spmd(nc, in_maps, list(range(NCORES)))
    return np.concatenate([res.results[c]["out"] for c in range(NCORES)], axis=0)
